# revision 32
# baseline (speedup 1.0000x reference)
"""ConvLSTM (pixel-wise, 1x1 convs) Trainium2 Bass kernel.

Math (after exact algebraic folding):
  per pixel, per t:  g1 = W1x @ x_t + W1h @ h1 + b1   (W1x = Wih1 @ (W_red * denorm_scale))
                     i,f,g,o = split(g1); c1 = sig(f)*c1 + sig(i)*tanh(g); h1 = sig(o)*tanh(c1)
                     g2 = W21 @ h1 + W22 @ h2 + b2    (W21 = Wih2 @ Wc1)
                     c2,h2 analogous
  out = (W_head @ Wc2) @ h2_final + const

Sharding: batch b -> core b (8 cores, no collectives).

Per-core layout (one chunk = all 16384 pixels):
  S1e/S1o [128, HW] bf16 (alternating t): rows 0:64 h1, rows 64:92 x(t),
      row 92 ones (shipped as a 29th x channel in the DMA). One K=93
      matmul per gate computes W1h@h1 + W1x@x + b1; zero copies.
  S2 [128, HW] bf16: rows 64:128 h2. Cell2 gates accumulate two
      row-group-DISJOINT matmuls: W21 @ S1next[0:64] (groups 0,1) then
      W22 @ S2[64:128] (groups 2,3) -- LDWEIGHTS overlaps, no PE bubbles.
  c1/c2 [128, HALF]: A-half pixels on partitions 0:64, B-half on 64:128.
  Gates per cell land in ONE [128, 4F] PSUM tile in order (i, f, o, g):
      one merged Sigmoid over [128, 3F] + one Tanh over [128, F]
      (ACT inst fixed cost is ~525 cyc -- minimize instruction count).
  Emission is software-pipelined across groups so the ACT queue is fed
  continuously: slot s runs {act1(s), C1-mms(s+1), dve1(s), act2(s-1),
  tanh_c1(s), dve2(s-1), h1(s), tanh_c2(s-1), C2-mms(s), h2(s-1)}.
"""

import numpy as np
import ml_dtypes

import concourse.bass as bass
import concourse.tile as tile
from concourse import bacc, mybir
from concourse.bass_utils import run_bass_kernel_spmd

F32 = mybir.dt.float32
BF16 = mybir.dt.bfloat16
AF = mybir.ActivationFunctionType

T, CIN, HID = 8, 28, 64
H = W = 128
HW = H * W            # pixels per core (one batch element)
HALF = HW // 2
NCORES = 8
K1 = HID + CIN + 1    # 93: h1 rows, x rows, ones row

import os
CFG = dict(
    fd=512,            # pixels per half per group (psum: 2 cells x [128, 4*fd] fp32)
    c_dtype="f32",     # c-state dtype: "f32" | "bf16"
    plane_bufs=2,
    emit="pipe",       # "pipe" (software-pipelined slots) | "serial" (per-group)
    gmode="vtanh",     # "vtanh": sigma(x)=(tanh(x/2)+1)/2 -> ONE merged Tanh
                       # over all 4 gates, doubled c/h state folded into
                       # weights (bf16-safe: only tanh values are stored);
                       # "tanh": exact sigmoid+tanh on separate insts
    vstt=7,            # vtanh bisect bits: 1=h_muls STT, 2=dve_c STT, 4=merged act
    warmup=0,          # PE warmup matmul count (HAM clock-gate priming)
)
for _k in list(CFG):
    _v = os.environ.get(f"KCFG_{_k.upper()}")
    if _v is not None:
        CFG[_k] = int(_v) if _v.isdigit() else _v


def _fold_weights(inputs):
    """Host-side exact algebraic folding (all fp32 numpy)."""
    f = np.float32
    W_red = inputs["W_red"].astype(f)
    b_red = inputs["b_red"].astype(f)
    # de-normalization of channels 11 (u) and 12 (v), folded into W_red
    a = np.ones(CIN, f); a[11] = f(0.15); a[12] = f(0.12)
    d = np.zeros(CIN, f); d[11] = f(0.02); d[12] = f(-0.01)
    W_red_eff = W_red * a[None, :]
    b_red_eff = b_red + W_red @ d

    W1x = inputs["Wih1"].astype(f) @ W_red_eff          # [256, 28]
    W1h = inputs["Whh1"].astype(f)                      # [256, 64]
    b1 = (inputs["bih1"] + inputs["bhh1"]).astype(f) + inputs["Wih1"].astype(f) @ b_red_eff
    W21 = inputs["Wih2"].astype(f) @ inputs["Wc1"].astype(f)   # [256, 64]
    W22 = inputs["Whh2"].astype(f)                      # [256, 64]
    b2 = (inputs["bih2"] + inputs["bhh2"]).astype(f) + inputs["Wih2"].astype(f) @ inputs["bc1"].astype(f)
    whead = (inputs["W_head"].astype(f) @ inputs["Wc2"].astype(f))[0]     # [64]
    bhead = float((inputs["W_head"].astype(f) @ inputs["bc2"].astype(f) + inputs["b_head"].astype(f)).reshape(()))

    # reorder gate blocks (i, f, g, o) -> (i, f, o, g) so the three
    # sigmoid gates are contiguous in the PSUM tile
    perm = np.r_[0:64, 64:128, 192:256, 128:192]
    W1x, W1h, W21, W22 = W1x[perm], W1h[perm], W21[perm], W22[perm]
    b1, b2 = b1[perm], b2[perm]


    w1 = np.zeros((128, 256), f)
    w1[0:HID] = W1h.T
    w1[HID:HID + CIN] = W1x.T
    w1[HID + CIN] = b1
    w2a = np.zeros((128, 256), f)
    w2a[0:HID] = W21.T
    w2b = np.zeros((128, 256), f)
    w2b[HID:128] = W22.T
    # cell2 bias rides an extra K=29 matmul against [x; ones] only when
    # nonzero (it is zero for the reference weight distribution)
    w2c = None
    if np.any(b2 != 0):
        w2c = np.zeros((128, 256), f)
        w2c[HID + CIN] = b2
    wh = np.zeros((128, 1), f)
    wh[HID:, 0] = whead
    d = dict(w1=w1, w2a=w2a, w2b=w2b, wh=wh)
    if w2c is not None:
        d["w2c"] = w2c
    if CFG["gmode"] == "vtanh":
        # sigma(x) = (tanh(x/2) + 1)/2: halve the i,f,o gate pre-activations
        # (cols 0:192) so ONE Tanh inst covers all four gates; h is stored
        # doubled (h_hat = (tanh(o-pre)+1)*tanh(c)) so halve every weight row
        # that reads it; c is stored doubled too (ACT tanh(c) uses scale=0.5).
        for nm, M in d.items():
            if nm != "wh":
                M[:, 0:192] *= 0.5
        d["w1"][0:HID] *= 0.5     # rows reading h1_hat
        d["w2a"][0:HID] *= 0.5
        d["w2b"][HID:128] *= 0.5  # rows reading h2_hat
        d["wh"] = wh * 0.5
    return d, bhead


def build(nc, bhead, has_b2):
    fd = CFG["fd"]
    ngrp = HALF // fd
    NSLOT = T * ngrp
    st_dt = {"f32": F32, "bf16": BF16}[CFG["c_dtype"]]

    x_d = nc.dram_tensor("xt", [T, CIN + 1, HW], BF16, kind="ExternalInput").ap()
    w_names = ["w1", "w2a", "w2b", "wh"] + (["w2c"] if has_b2 else [])
    w_dram = {nm: nc.dram_tensor(nm, [128, 1] if nm == "wh" else [128, 256], F32,
                                 kind="ExternalInput").ap() for nm in w_names}
    # out[i, j] = pixel j*128 + i of this core's [H, W] map (host transposes)
    out_d = nc.dram_tensor("out", [128, HW // 128], F32, kind="ExternalOutput").ap()

    with tile.TileContext(nc) as tc:
        with (
            tc.tile_pool(name="const", bufs=1) as const,
            tc.tile_pool(name="state", bufs=1) as state,
            tc.tile_pool(name="planes", bufs=CFG["plane_bufs"]) as planes,
            tc.tile_pool(name="outp", bufs=1) as outp,
            tc.tile_pool(name="psum", bufs=1, space=bass.MemorySpace.PSUM) as psum,
        ):
            # Stage weights via fp32 tiles + one convert copy each, so every
            # matmul waits on a single compute producer (the fused LDWEIGHTS
            # has very few sync-wait slots).
            w_sb = {}
            for nm in w_names:
                shp = [128, 1] if nm == "wh" else [128, 256]
                wf = const.tile(shp, F32, tag=f"{nm}f")
                nc.sync.dma_start(wf[:], w_dram[nm])
                wb = const.tile(shp, BF16, tag=nm)
                nc.vector.tensor_copy(wb[:], wf[:])
                w_sb[nm] = wb

            S1e = state.tile([128, HW], BF16, tag="S1e")
            S1o = state.tile([128, HW], BF16, tag="S1o")
            Ss = [S1e, S1o]
            S2 = state.tile([128, HW], BF16, tag="S2")
            c1 = state.tile([128, HALF], st_dt, tag="c1")
            c2 = state.tile([128, HALF], st_dt, tag="c2")
            out_sb = outp.tile([128, HW // 128], F32, tag="osb")

            # PE warmup: ~4.5us of back-to-back matmuls so the PE_HAM clock
            # gate opens (cold 1.2 GHz -> warm 2.4 GHz) before the real work;
            # steady-state PE gaps stay under the ~3.4us re-throttle window.
            if CFG["warmup"]:
                wu = psum.tile([128, 4 * fd], F32, tag="P0", name="warm")
                for _ in range(CFG["warmup"]):
                    nc.tensor.matmul(wu[0:64, 0:128],
                                     w_sb["w1"][0:HID, 0:64],
                                     w_sb["w1"][0:HID, 0:128])

            # per-slot live tile handles (psum gate tiles + act output planes)
            P0s, P1s, pl1, pl2 = {}, {}, {}, {}

            def slot_tg(s):
                return s // ngrp, s % ngrp

            def cols(g):
                return g * fd, HALF + g * fd      # A-half / B-half col starts

            def c1_mms(s):
                if not (0 <= s < NSLOT):
                    return
                t, g = slot_tg(s)
                Scur = Ss[t % 2]
                if g == 0:
                    if t == 0:
                        nc.sync.dma_start(Scur[HID:K1, :], x_d[0])
                    if t + 1 < T:
                        nc.sync.dma_start(Ss[(t + 1) % 2][HID:K1, :], x_d[t + 1])
                a0, b0 = cols(g)
                ks = slice(0, K1) if t > 0 else slice(HID, K1)
                P = psum.tile([128, 4 * fd], F32, tag="P0", name="P0")
                P0s[s] = P
                for q in range(4):
                    for (cb, po) in ((a0, 0), (b0, 64)):
                        nc.tensor.matmul(
                            P[po:po + 64, q * fd:(q + 1) * fd],
                            w_sb["w1"][ks, q * 64:(q + 1) * 64],
                            Scur[ks, cb:cb + fd],
                        )

            def act_gates(s, Ps, pl, tagp):
                if not (0 <= s < NSLOT):
                    return
                P = Ps.pop(s)
                if CFG["gmode"] == "vtanh":
                    sfo = planes.tile([128, 4 * fd], BF16, tag=f"sfo{tagp}")
                    if CFG["vstt"] & 4:
                        nc.scalar.activation(sfo[:], P[:], AF.Tanh)
                    else:
                        nc.scalar.activation(sfo[:, 0:3 * fd], P[:, 0:3 * fd], AF.Tanh)
                        nc.scalar.activation(sfo[:, 3 * fd:4 * fd], P[:, 3 * fd:4 * fd], AF.Tanh)
                    pl[s] = (sfo, sfo[:, 3 * fd:4 * fd])
                else:
                    sfo = planes.tile([128, 3 * fd], BF16, tag=f"sfo{tagp}")
                    tgp = planes.tile([128, fd], BF16, tag=f"tg{tagp}")
                    nc.scalar.activation(sfo[:], P[:, 0:3 * fd], AF.Sigmoid)
                    nc.scalar.activation(tgp[:], P[:, 3 * fd:4 * fd], AF.Tanh)
                    pl[s] = (sfo, tgp[:])

            def act1(s):
                act_gates(s, P0s, pl1, "1")

            def dve_c(s, pl, cc, tagp):
                # gmode=vtanh (v* = tanh(pre/2), c_hat = 2c):
                #   c_hat = (v_f+1)*c_hat*0.5 + (v_i+1)*tanh(g)
                # gmode=tanh: c = sig(f)*c + sig(i)*tanh(g)
                if not (0 <= s < NSLOT):
                    return
                t, g = slot_tg(s)
                sfo, tgp = pl[s]
                cg = slice(g * fd, (g + 1) * fd)
                si = sfo[:, 0:fd]
                sf = sfo[:, fd:2 * fd]
                Alu = mybir.AluOpType
                vt = CFG["gmode"] == "vtanh"
                if vt:
                    if CFG["vstt"] & 2:
                        if t > 0:
                            p = planes.tile([128, fd], BF16, tag=f"t2{tagp}")
                            q = planes.tile([128, fd], st_dt, tag=f"t1{tagp}")
                            nc.vector.scalar_tensor_tensor(
                                p[:], si, 1.0, tgp, Alu.add, Alu.mult)
                            nc.vector.scalar_tensor_tensor(
                                q[:], sf, 1.0, cc[:, cg], Alu.add, Alu.mult)
                            nc.vector.scalar_tensor_tensor(
                                cc[:, cg], q[:], 0.5, p[:], Alu.mult, Alu.add)
                        else:
                            nc.vector.scalar_tensor_tensor(
                                cc[:, cg], si, 1.0, tgp, Alu.add, Alu.mult)
                    else:
                        p = planes.tile([128, fd], BF16, tag=f"t2{tagp}")
                        ti = planes.tile([128, fd], BF16, tag=f"ti{tagp}")
                        nc.vector.tensor_scalar_add(ti[:], si, 1.0)
                        nc.vector.tensor_mul(p[:], ti[:], tgp)
                        if t > 0:
                            q = planes.tile([128, fd], st_dt, tag=f"t1{tagp}")
                            tf = planes.tile([128, fd], BF16, tag=f"tf{tagp}")
                            nc.vector.tensor_scalar_add(tf[:], sf, 1.0)
                            nc.vector.tensor_mul(q[:], tf[:], cc[:, cg])
                            qh = planes.tile([128, fd], st_dt, tag=f"qh{tagp}")
                            nc.vector.tensor_scalar_mul(qh[:], q[:], 0.5)
                            nc.vector.tensor_add(cc[:, cg], qh[:], p[:])
                        else:
                            nc.vector.tensor_copy(cc[:, cg], p[:])
                    return
                t2 = planes.tile([128, fd], BF16, tag=f"t2{tagp}")
                if t > 0:
                    t1 = planes.tile([128, fd], st_dt, tag=f"t1{tagp}")
                    nc.vector.tensor_mul(t2[:], si, tgp)
                    nc.vector.tensor_mul(t1[:], sf, cc[:, cg])
                    nc.vector.tensor_add(cc[:, cg], t1[:], t2[:])
                else:
                    nc.vector.tensor_mul(cc[:, cg], si, tgp)

            def act_tc(s, pl, cc, tagp):
                if not (0 <= s < NSLOT):
                    return
                t, g = slot_tg(s)
                cg = slice(g * fd, (g + 1) * fd)
                tch = planes.tile([128, fd], BF16, tag=f"tch{tagp}")
                scl = 0.5 if CFG["gmode"] == "vtanh" else 1.0
                nc.scalar.activation(tch[:], cc[:, cg], AF.Tanh, scale=scl)
                pl[s] = (pl[s][0], tch)

            def h_muls(s, pl, dst_tile_fn, dst_rows):
                # h = sig(o)*tanh(c); vtanh stores h_hat = (v_o+1)*tanh(c) = 2h
                if not (0 <= s < NSLOT):
                    return
                t, g = slot_tg(s)
                sfo, tch = pl.pop(s)
                a0, b0 = cols(g)
                dst = dst_tile_fn(t)
                Alu = mybir.AluOpType
                for (po, cb) in ((0, a0), (64, b0)):
                    so = sfo[po:po + 64, 2 * fd:3 * fd]
                    if CFG["gmode"] == "vtanh":
                        if CFG["vstt"] & 1:
                            nc.vector.scalar_tensor_tensor(
                                dst[dst_rows, cb:cb + fd], so, 1.0,
                                tch[po:po + 64, :], Alu.add, Alu.mult)
                        else:
                            to = planes.tile([64, fd], BF16, tag=f"to{po}")
                            nc.vector.tensor_scalar_add(to[:], so, 1.0)
                            nc.vector.tensor_mul(dst[dst_rows, cb:cb + fd],
                                                 to[:], tch[po:po + 64, :])
                    else:
                        nc.vector.tensor_mul(dst[dst_rows, cb:cb + fd],
                                             so, tch[po:po + 64, :])

            def c2_mms(s):
                if not (0 <= s < NSLOT):
                    return
                t, g = slot_tg(s)
                Snxt = Ss[(t + 1) % 2]
                a0, b0 = cols(g)
                P = psum.tile([128, 4 * fd], F32, tag="P1", name="P1")
                P1s[s] = P
                # Per gate: K=65 rows 0:65 (h1 + a zero weight row) opens the
                # accumulation, K=64 rows 64:128 (h2) closes it. The K=65 pad
                # is deliberate: a K<=64 pair with bases 0 and 64 into one
                # PSUM region makes walrus pick row-tiling mode, and a row-
                # tiled mm accumulating onto another row tile's output faults
                # the HW (minimal repro verified). K only shifts the drain, so
                # the pad is free.
                # Waves: all four "a" mms stream back-to-back (same rows, so
                # LDWEIGHTS pulls ahead) on alternating column halves (col-
                # tiled pairs run concurrently), then the "b" wave closes the
                # groups; a second wave pass covers the swapped halves. Only
                # one accumulation group is ever pending per PSUM bank.
                halves = ((a0, 0), (b0, 64))
                for wave in range(2):
                    for q in range(4):
                        cb, po = halves[(q + wave) % 2]
                        nc.tensor.matmul(
                            P[po:po + 64, q * fd:(q + 1) * fd],
                            w_sb["w2a"][0:HID + 1, q * 64:(q + 1) * 64],
                            Snxt[0:HID + 1, cb:cb + fd],
                            start=True, stop=(t == 0 and not has_b2),
                        )
                    for q in range(4):
                        cb, po = halves[(q + wave) % 2]
                        if t > 0:
                            nc.tensor.matmul(
                                P[po:po + 64, q * fd:(q + 1) * fd],
                                w_sb["w2b"][HID:128, q * 64:(q + 1) * 64],
                                S2[HID:128, cb:cb + fd],
                                start=False, stop=not has_b2,
                            )
                        if has_b2:
                            nc.tensor.matmul(
                                P[po:po + 64, q * fd:(q + 1) * fd],
                                w_sb["w2c"][HID:K1, q * 64:(q + 1) * 64],
                                Snxt[HID:K1, cb:cb + fd],
                                start=False, stop=True,
                            )

            def act2(s):
                act_gates(s, P1s, pl2, "2")

            s1next = lambda t: Ss[(t + 1) % 2]
            s2tile = lambda t: S2

            if CFG["emit"] == "pipe":
                # Software pipeline, cell2 lagged two slots behind cell1's
                # matmuls: C2(s-1) runs at slot-s start with h1(s-1) already
                # computed last slot, so the PE never waits on the
                # act->dve->tanh->h chain inside a slot.
                c1_mms(0)
                for s in range(NSLOT + 2):
                    act1(s)
                    c1_mms(s + 1)
                    dve_c(s, pl1, c1, "a")
                    act2(s - 2)
                    act_tc(s, pl1, c1, "a")
                    dve_c(s - 2, pl2, c2, "b")
                    h_muls(s, pl1, s1next, slice(0, HID))
                    act_tc(s - 2, pl2, c2, "b")
                    c2_mms(s - 1)
                    h_muls(s - 2, pl2, s2tile, slice(HID, 128))
            else:
                for s in range(NSLOT):
                    c1_mms(s)
                    act1(s)
                    dve_c(s, pl1, c1, "a")
                    act_tc(s, pl1, c1, "a")
                    h_muls(s, pl1, s1next, slice(0, HID))
                    c2_mms(s)
                    act2(s)
                    dve_c(s, pl2, c2, "b")
                    act_tc(s, pl2, c2, "b")
                    h_muls(s, pl2, s2tile, slice(HID, 128))

            # head: out[pix] = whead @ h2[pix] + bhead, pixels as matmul M-dim
            ncols = HW // 128
            ph = psum.tile([128, ncols], F32, tag="P0", name="ph")
            for j in range(ncols):
                nc.tensor.matmul(
                    ph[:, j:j + 1],
                    S2[HID:128, j * 128:(j + 1) * 128],
                    w_sb["wh"][HID:128, 0:1],
                )
            nc.vector.tensor_scalar_add(out_sb[:], ph[:], float(bhead))
            nc.sync.dma_start(out_d, out_sb[:])
    nc.compile()
    return nc


def _make_nc():
    # Bacc (not raw Bass): its compile() runs move_matmul_waits_to_ldweights +
    # generate_event_semaphores, required to satisfy TRN2's 1-wait-per-inst limit.
    return bacc.Bacc("TRN2", target_bir_lowering=False, debug=False,
                     num_devices=NCORES, enable_partition_id=False)


def _in_maps(inputs):
    folded, _ = _fold_weights(inputs)
    x = np.asarray(inputs["x"], dtype=np.float32)
    x_bf = x.reshape(NCORES, T, CIN, HW).astype(ml_dtypes.bfloat16)
    ones = np.ones((T, 1, HW), ml_dtypes.bfloat16)
    maps = []
    for b in range(NCORES):
        m = dict(folded)
        m["xt"] = np.ascontiguousarray(
            np.concatenate([x_bf[b], ones], axis=1))
        maps.append(m)
    return maps


def _assemble(results):
    out = np.empty((NCORES, H, W), np.float32)
    for b in range(NCORES):
        o = results[b]["out"]          # [128, HW//128], o[i, j] = pixel j*128+i
        out[b] = o.T.reshape(H, W)
    return out


def _run(inputs, trace=False):
    folded, bhead = _fold_weights(inputs)
    nc = build(_make_nc(), bhead, "w2c" in folded)
    maps = _in_maps(inputs)
    res = run_bass_kernel_spmd(nc, maps, core_ids=list(range(NCORES)), trace=trace)
    return _assemble(res.results), res


def kernel(**inputs) -> np.ndarray:
    out, _ = _run(inputs, trace=False)
    return out


# revision 34
# speedup vs baseline: 1.0362x; 1.0362x over previous
"""ConvLSTM (pixel-wise, 1x1 convs) Trainium2 Bass kernel.

Math (after exact algebraic folding):
  per pixel, per t:  g1 = W1x @ x_t + W1h @ h1 + b1   (W1x = Wih1 @ (W_red * denorm_scale))
                     i,f,g,o = split(g1); c1 = sig(f)*c1 + sig(i)*tanh(g); h1 = sig(o)*tanh(c1)
                     g2 = W21 @ h1 + W22 @ h2 + b2    (W21 = Wih2 @ Wc1)
                     c2,h2 analogous
  out = (W_head @ Wc2) @ h2_final + const

Sharding: batch b -> core b (8 cores, no collectives).

Per-core layout (one chunk = all 16384 pixels):
  S1e/S1o [128, HW] bf16 (alternating t): rows 0:64 h1, rows 64:92 x(t),
      row 92 ones (shipped as a 29th x channel in the DMA). One K=93
      matmul per gate computes W1h@h1 + W1x@x + b1; zero copies.
  S2 [128, HW] bf16: rows 64:128 h2. Cell2 gates accumulate two
      row-group-DISJOINT matmuls: W21 @ S1next[0:64] (groups 0,1) then
      W22 @ S2[64:128] (groups 2,3) -- LDWEIGHTS overlaps, no PE bubbles.
  c1/c2 [128, HALF]: A-half pixels on partitions 0:64, B-half on 64:128.
  Gates per cell land in ONE [128, 4F] PSUM tile in order (i, f, o, g):
      one merged Sigmoid over [128, 3F] + one Tanh over [128, F]
      (ACT inst fixed cost is ~525 cyc -- minimize instruction count).
  Emission is software-pipelined across groups so the ACT queue is fed
  continuously: slot s runs {act1(s), C1-mms(s+1), dve1(s), act2(s-1),
  tanh_c1(s), dve2(s-1), h1(s), tanh_c2(s-1), C2-mms(s), h2(s-1)}.
"""

import numpy as np
import ml_dtypes

import concourse.bass as bass
import concourse.tile as tile
from concourse import bacc, mybir
from concourse.bass_utils import run_bass_kernel_spmd

F32 = mybir.dt.float32
BF16 = mybir.dt.bfloat16
AF = mybir.ActivationFunctionType

T, CIN, HID = 8, 28, 64
H = W = 128
HW = H * W            # pixels per core (one batch element)
HALF = HW // 2
NCORES = 8
K1 = HID + CIN + 1    # 93: h1 rows, x rows, ones row

import os
CFG = dict(
    fd=512,            # pixels per half per group (psum: 2 cells x [128, 4*fd] fp32)
    c_dtype="f32",     # c-state dtype: "f32" | "bf16"
    plane_bufs=2,
    emit="pipe",       # "pipe" (software-pipelined slots) | "serial" (per-group)
    gmode="vtanh",     # "vtanh": sigma(x)=(tanh(x/2)+1)/2 -> ONE merged Tanh
                       # over all 4 gates, doubled c/h state folded into
                       # weights (bf16-safe: only tanh values are stored);
                       # "tanh": exact sigmoid+tanh on separate insts
    vstt=7,            # vtanh bisect bits: 1=h_muls STT, 2=dve_c STT, 4=merged act
    warmup=0,          # PE warmup matmul count (HAM clock-gate priming)
)
for _k in list(CFG):
    _v = os.environ.get(f"KCFG_{_k.upper()}")
    if _v is not None:
        CFG[_k] = int(_v) if _v.isdigit() else _v


def _fold_weights(inputs):
    """Host-side exact algebraic folding (all fp32 numpy)."""
    f = np.float32
    W_red = inputs["W_red"].astype(f)
    b_red = inputs["b_red"].astype(f)
    # de-normalization of channels 11 (u) and 12 (v), folded into W_red
    a = np.ones(CIN, f); a[11] = f(0.15); a[12] = f(0.12)
    d = np.zeros(CIN, f); d[11] = f(0.02); d[12] = f(-0.01)
    W_red_eff = W_red * a[None, :]
    b_red_eff = b_red + W_red @ d

    W1x = inputs["Wih1"].astype(f) @ W_red_eff          # [256, 28]
    W1h = inputs["Whh1"].astype(f)                      # [256, 64]
    b1 = (inputs["bih1"] + inputs["bhh1"]).astype(f) + inputs["Wih1"].astype(f) @ b_red_eff
    W21 = inputs["Wih2"].astype(f) @ inputs["Wc1"].astype(f)   # [256, 64]
    W22 = inputs["Whh2"].astype(f)                      # [256, 64]
    b2 = (inputs["bih2"] + inputs["bhh2"]).astype(f) + inputs["Wih2"].astype(f) @ inputs["bc1"].astype(f)
    whead = (inputs["W_head"].astype(f) @ inputs["Wc2"].astype(f))[0]     # [64]
    bhead = float((inputs["W_head"].astype(f) @ inputs["bc2"].astype(f) + inputs["b_head"].astype(f)).reshape(()))

    # reorder gate blocks (i, f, g, o) -> (i, f, o, g) so the three
    # sigmoid gates are contiguous in the PSUM tile
    perm = np.r_[0:64, 64:128, 192:256, 128:192]
    W1x, W1h, W21, W22 = W1x[perm], W1h[perm], W21[perm], W22[perm]
    b1, b2 = b1[perm], b2[perm]


    w1 = np.zeros((128, 256), f)
    w1[0:HID] = W1h.T
    w1[HID:HID + CIN] = W1x.T
    w1[HID + CIN] = b1
    w2a = np.zeros((128, 256), f)
    w2a[0:HID] = W21.T
    w2b = np.zeros((128, 256), f)
    w2b[HID:128] = W22.T
    # cell2 bias rides an extra K=29 matmul against [x; ones] only when
    # nonzero (it is zero for the reference weight distribution)
    w2c = None
    if np.any(b2 != 0):
        w2c = np.zeros((128, 256), f)
        w2c[HID + CIN] = b2
    wh = np.zeros((128, 1), f)
    wh[HID:, 0] = whead
    d = dict(w1=w1, w2a=w2a, w2b=w2b, wh=wh)
    if w2c is not None:
        d["w2c"] = w2c
    if CFG["gmode"] == "vtanh":
        # sigma(x) = (tanh(x/2) + 1)/2: halve the i,f,o gate pre-activations
        # (cols 0:192) so ONE Tanh inst covers all four gates; h is stored
        # doubled (h_hat = (tanh(o-pre)+1)*tanh(c)) so halve every weight row
        # that reads it; c is stored doubled too (ACT tanh(c) uses scale=0.5).
        for nm, M in d.items():
            if nm != "wh":
                M[:, 0:192] *= 0.5
        d["w1"][0:HID] *= 0.5     # rows reading h1_hat
        d["w2a"][0:HID] *= 0.5
        d["w2b"][HID:128] *= 0.5  # rows reading h2_hat
        d["wh"] = wh * 0.5
    return d, bhead


def build(nc, bhead, has_b2):
    fd = CFG["fd"]
    ngrp = HALF // fd
    NSLOT = T * ngrp
    st_dt = {"f32": F32, "bf16": BF16}[CFG["c_dtype"]]

    x_d = nc.dram_tensor("xt", [T, CIN + 1, HW], BF16, kind="ExternalInput").ap()
    w_names = ["w1", "w2a", "w2b", "wh"] + (["w2c"] if has_b2 else [])
    w_dram = {nm: nc.dram_tensor(nm, [128, 1] if nm == "wh" else [128, 256], F32,
                                 kind="ExternalInput").ap() for nm in w_names}
    # out[i, j] = pixel j*128 + i of this core's [H, W] map (host transposes)
    out_d = nc.dram_tensor("out", [128, HW // 128], F32, kind="ExternalOutput").ap()

    with tile.TileContext(nc) as tc:
        with (
            tc.tile_pool(name="const", bufs=1) as const,
            tc.tile_pool(name="state", bufs=1) as state,
            tc.tile_pool(name="planes", bufs=CFG["plane_bufs"]) as planes,
            tc.tile_pool(name="outp", bufs=1) as outp,
            tc.tile_pool(name="psum", bufs=1, space=bass.MemorySpace.PSUM) as psum,
        ):
            # Stage weights via fp32 tiles + one convert copy each, so every
            # matmul waits on a single compute producer (the fused LDWEIGHTS
            # has very few sync-wait slots).
            w_sb = {}
            for nm in w_names:
                shp = [128, 1] if nm == "wh" else [128, 256]
                wf = const.tile(shp, F32, tag=f"{nm}f")
                nc.sync.dma_start(wf[:], w_dram[nm])
                wb = const.tile(shp, BF16, tag=nm)
                nc.vector.tensor_copy(wb[:], wf[:])
                w_sb[nm] = wb

            S1e = state.tile([128, HW], BF16, tag="S1e")
            S1o = state.tile([128, HW], BF16, tag="S1o")
            Ss = [S1e, S1o]
            S2 = state.tile([128, HW], BF16, tag="S2")
            c1 = state.tile([128, HALF], st_dt, tag="c1")
            c2 = state.tile([128, HALF], st_dt, tag="c2")
            out_sb = outp.tile([128, HW // 128], F32, tag="osb")

            # PE warmup: ~4.5us of back-to-back matmuls so the PE_HAM clock
            # gate opens (cold 1.2 GHz -> warm 2.4 GHz) before the real work;
            # steady-state PE gaps stay under the ~3.4us re-throttle window.
            if CFG["warmup"]:
                wu = psum.tile([128, 4 * fd], F32, tag="P0", name="warm")
                for _ in range(CFG["warmup"]):
                    nc.tensor.matmul(wu[0:64, 0:128],
                                     w_sb["w1"][0:HID, 0:64],
                                     w_sb["w1"][0:HID, 0:128])

            # per-slot live tile handles (psum gate tiles + act output planes)
            P0s, P1s, pl1, pl2 = {}, {}, {}, {}

            def slot_tg(s):
                return s // ngrp, s % ngrp

            def cols(g):
                return g * fd, HALF + g * fd      # A-half / B-half col starts

            def c1_mms(s):
                if not (0 <= s < NSLOT):
                    return
                t, g = slot_tg(s)
                Scur = Ss[t % 2]
                if g == 0:
                    if t == 0:
                        nc.sync.dma_start(Scur[HID:K1, :], x_d[0])
                    if t + 1 < T:
                        nc.sync.dma_start(Ss[(t + 1) % 2][HID:K1, :], x_d[t + 1])
                a0, b0 = cols(g)
                ks = slice(0, K1) if t > 0 else slice(HID, K1)
                P = psum.tile([128, 4 * fd], F32, tag="P0", name="P0")
                P0s[s] = P
                for q in range(4):
                    for (cb, po) in ((a0, 0), (b0, 64)):
                        nc.tensor.matmul(
                            P[po:po + 64, q * fd:(q + 1) * fd],
                            w_sb["w1"][ks, q * 64:(q + 1) * 64],
                            Scur[ks, cb:cb + fd],
                        )

            def act_gates(s, Ps, pl, tagp):
                if not (0 <= s < NSLOT):
                    return
                P = Ps.pop(s)
                if CFG["gmode"] == "vtanh":
                    sfo = planes.tile([128, 4 * fd], BF16, tag=f"sfo{tagp}")
                    if CFG["vstt"] & 4:
                        nc.scalar.activation(sfo[:], P[:], AF.Tanh)
                    else:
                        nc.scalar.activation(sfo[:, 0:3 * fd], P[:, 0:3 * fd], AF.Tanh)
                        nc.scalar.activation(sfo[:, 3 * fd:4 * fd], P[:, 3 * fd:4 * fd], AF.Tanh)
                    pl[s] = (sfo, sfo[:, 3 * fd:4 * fd])
                else:
                    sfo = planes.tile([128, 3 * fd], BF16, tag=f"sfo{tagp}")
                    tgp = planes.tile([128, fd], BF16, tag=f"tg{tagp}")
                    nc.scalar.activation(sfo[:], P[:, 0:3 * fd], AF.Sigmoid)
                    nc.scalar.activation(tgp[:], P[:, 3 * fd:4 * fd], AF.Tanh)
                    pl[s] = (sfo, tgp[:])

            def act1(s):
                act_gates(s, P0s, pl1, "1")

            def dve_c(s, pl, cc, tagp):
                # gmode=vtanh (v* = tanh(pre/2), c_hat = 2c):
                #   c_hat = (v_f+1)*c_hat*0.5 + (v_i+1)*tanh(g)
                # gmode=tanh: c = sig(f)*c + sig(i)*tanh(g)
                if not (0 <= s < NSLOT):
                    return
                t, g = slot_tg(s)
                sfo, tgp = pl[s]
                cg = slice(g * fd, (g + 1) * fd)
                si = sfo[:, 0:fd]
                sf = sfo[:, fd:2 * fd]
                Alu = mybir.AluOpType
                vt = CFG["gmode"] == "vtanh"
                if vt:
                    if CFG["vstt"] & 2:
                        if t > 0:
                            p = planes.tile([128, fd], BF16, tag=f"t2{tagp}")
                            q = planes.tile([128, fd], st_dt, tag=f"t1{tagp}")
                            nc.vector.scalar_tensor_tensor(
                                p[:], si, 1.0, tgp, Alu.add, Alu.mult)
                            nc.vector.scalar_tensor_tensor(
                                q[:], sf, 1.0, cc[:, cg], Alu.add, Alu.mult)
                            nc.vector.scalar_tensor_tensor(
                                cc[:, cg], q[:], 0.5, p[:], Alu.mult, Alu.add)
                        else:
                            nc.vector.scalar_tensor_tensor(
                                cc[:, cg], si, 1.0, tgp, Alu.add, Alu.mult)
                    else:
                        p = planes.tile([128, fd], BF16, tag=f"t2{tagp}")
                        ti = planes.tile([128, fd], BF16, tag=f"ti{tagp}")
                        nc.vector.tensor_scalar_add(ti[:], si, 1.0)
                        nc.vector.tensor_mul(p[:], ti[:], tgp)
                        if t > 0:
                            q = planes.tile([128, fd], st_dt, tag=f"t1{tagp}")
                            tf = planes.tile([128, fd], BF16, tag=f"tf{tagp}")
                            nc.vector.tensor_scalar_add(tf[:], sf, 1.0)
                            nc.vector.tensor_mul(q[:], tf[:], cc[:, cg])
                            qh = planes.tile([128, fd], st_dt, tag=f"qh{tagp}")
                            nc.vector.tensor_scalar_mul(qh[:], q[:], 0.5)
                            nc.vector.tensor_add(cc[:, cg], qh[:], p[:])
                        else:
                            nc.vector.tensor_copy(cc[:, cg], p[:])
                    return
                t2 = planes.tile([128, fd], BF16, tag=f"t2{tagp}")
                if t > 0:
                    t1 = planes.tile([128, fd], st_dt, tag=f"t1{tagp}")
                    nc.vector.tensor_mul(t2[:], si, tgp)
                    nc.vector.tensor_mul(t1[:], sf, cc[:, cg])
                    nc.vector.tensor_add(cc[:, cg], t1[:], t2[:])
                else:
                    nc.vector.tensor_mul(cc[:, cg], si, tgp)

            def act_tc(s, pl, cc, tagp):
                if not (0 <= s < NSLOT):
                    return
                t, g = slot_tg(s)
                cg = slice(g * fd, (g + 1) * fd)
                tch = planes.tile([128, fd], BF16, tag=f"tch{tagp}")
                scl = 0.5 if CFG["gmode"] == "vtanh" else 1.0
                nc.scalar.activation(tch[:], cc[:, cg], AF.Tanh, scale=scl)
                pl[s] = (pl[s][0], tch)

            def h_muls(s, pl, dst_tile_fn, dst_rows, tagp):
                # h = sig(o)*tanh(c); vtanh stores h_hat = (v_o+1)*tanh(c) = 2h
                if not (0 <= s < NSLOT):
                    return
                t, g = slot_tg(s)
                sfo, tch = pl.pop(s)
                a0, b0 = cols(g)
                dst = dst_tile_fn(t)
                Alu = mybir.AluOpType
                if CFG["gmode"] == "vtanh":
                    # one full-width STT then two 4x-mode bf16 copies into the
                    # state rows: 1 x 657ns + 2 x ~195ns beats 2 x 657ns STTs
                    hp = planes.tile([128, fd], BF16, tag=f"hp{tagp}")
                    nc.vector.scalar_tensor_tensor(
                        hp[:], sfo[:, 2 * fd:3 * fd], 1.0, tch[:],
                        Alu.add, Alu.mult)
                    nc.vector.tensor_copy(dst[dst_rows, a0:a0 + fd], hp[0:64, :])
                    nc.vector.tensor_copy(dst[dst_rows, b0:b0 + fd], hp[64:128, :])
                else:
                    for (po, cb) in ((0, a0), (64, b0)):
                        so = sfo[po:po + 64, 2 * fd:3 * fd]
                        nc.vector.tensor_mul(dst[dst_rows, cb:cb + fd],
                                             so, tch[po:po + 64, :])

            def c2_mms(s):
                if not (0 <= s < NSLOT):
                    return
                t, g = slot_tg(s)
                Snxt = Ss[(t + 1) % 2]
                a0, b0 = cols(g)
                P = psum.tile([128, 4 * fd], F32, tag="P1", name="P1")
                P1s[s] = P
                # Per gate: K=65 rows 0:65 (h1 + a zero weight row) opens the
                # accumulation, K=64 rows 64:128 (h2) closes it. The K=65 pad
                # is deliberate: a K<=64 pair with bases 0 and 64 into one
                # PSUM region makes walrus pick row-tiling mode, and a row-
                # tiled mm accumulating onto another row tile's output faults
                # the HW (minimal repro verified). K only shifts the drain, so
                # the pad is free.
                # Waves: all four "a" mms stream back-to-back (same rows, so
                # LDWEIGHTS pulls ahead) on alternating column halves (col-
                # tiled pairs run concurrently), then the "b" wave closes the
                # groups; a second wave pass covers the swapped halves. Only
                # one accumulation group is ever pending per PSUM bank.
                halves = ((a0, 0), (b0, 64))
                for wave in range(2):
                    for q in range(4):
                        cb, po = halves[(q + wave) % 2]
                        nc.tensor.matmul(
                            P[po:po + 64, q * fd:(q + 1) * fd],
                            w_sb["w2a"][0:HID + 1, q * 64:(q + 1) * 64],
                            Snxt[0:HID + 1, cb:cb + fd],
                            start=True, stop=(t == 0 and not has_b2),
                        )
                    for q in range(4):
                        cb, po = halves[(q + wave) % 2]
                        if t > 0:
                            nc.tensor.matmul(
                                P[po:po + 64, q * fd:(q + 1) * fd],
                                w_sb["w2b"][HID:128, q * 64:(q + 1) * 64],
                                S2[HID:128, cb:cb + fd],
                                start=False, stop=not has_b2,
                            )
                        if has_b2:
                            nc.tensor.matmul(
                                P[po:po + 64, q * fd:(q + 1) * fd],
                                w_sb["w2c"][HID:K1, q * 64:(q + 1) * 64],
                                Snxt[HID:K1, cb:cb + fd],
                                start=False, stop=True,
                            )

            def act2(s):
                act_gates(s, P1s, pl2, "2")

            s1next = lambda t: Ss[(t + 1) % 2]
            s2tile = lambda t: S2

            if CFG["emit"] == "pipe":
                # Software pipeline, cell2 lagged two slots behind cell1's
                # matmuls: C2(s-1) runs at slot-s start with h1(s-1) already
                # computed last slot, so the PE never waits on the
                # act->dve->tanh->h chain inside a slot.
                c1_mms(0)
                for s in range(NSLOT + 2):
                    act1(s)
                    c1_mms(s + 1)
                    dve_c(s, pl1, c1, "a")
                    act2(s - 2)
                    act_tc(s, pl1, c1, "a")
                    dve_c(s - 2, pl2, c2, "b")
                    h_muls(s, pl1, s1next, slice(0, HID), 'a')
                    act_tc(s - 2, pl2, c2, "b")
                    c2_mms(s - 1)
                    h_muls(s - 2, pl2, s2tile, slice(HID, 128), 'b')
            else:
                for s in range(NSLOT):
                    c1_mms(s)
                    act1(s)
                    dve_c(s, pl1, c1, "a")
                    act_tc(s, pl1, c1, "a")
                    h_muls(s, pl1, s1next, slice(0, HID), 'a')
                    c2_mms(s)
                    act2(s)
                    dve_c(s, pl2, c2, "b")
                    act_tc(s, pl2, c2, "b")
                    h_muls(s, pl2, s2tile, slice(HID, 128), 'b')

            # head: out[pix] = whead @ h2[pix] + bhead, pixels as matmul M-dim
            ncols = HW // 128
            ph = psum.tile([128, ncols], F32, tag="P0", name="ph")
            for j in range(ncols):
                nc.tensor.matmul(
                    ph[:, j:j + 1],
                    S2[HID:128, j * 128:(j + 1) * 128],
                    w_sb["wh"][HID:128, 0:1],
                )
            nc.vector.tensor_scalar_add(out_sb[:], ph[:], float(bhead))
            nc.sync.dma_start(out_d, out_sb[:])
    nc.compile()
    return nc


def _make_nc():
    # Bacc (not raw Bass): its compile() runs move_matmul_waits_to_ldweights +
    # generate_event_semaphores, required to satisfy TRN2's 1-wait-per-inst limit.
    return bacc.Bacc("TRN2", target_bir_lowering=False, debug=False,
                     num_devices=NCORES, enable_partition_id=False)


def _in_maps(inputs):
    folded, _ = _fold_weights(inputs)
    x = np.asarray(inputs["x"], dtype=np.float32)
    x_bf = x.reshape(NCORES, T, CIN, HW).astype(ml_dtypes.bfloat16)
    ones = np.ones((T, 1, HW), ml_dtypes.bfloat16)
    maps = []
    for b in range(NCORES):
        m = dict(folded)
        m["xt"] = np.ascontiguousarray(
            np.concatenate([x_bf[b], ones], axis=1))
        maps.append(m)
    return maps


def _assemble(results):
    out = np.empty((NCORES, H, W), np.float32)
    for b in range(NCORES):
        o = results[b]["out"]          # [128, HW//128], o[i, j] = pixel j*128+i
        out[b] = o.T.reshape(H, W)
    return out


def _run(inputs, trace=False):
    folded, bhead = _fold_weights(inputs)
    nc = build(_make_nc(), bhead, "w2c" in folded)
    maps = _in_maps(inputs)
    res = run_bass_kernel_spmd(nc, maps, core_ids=list(range(NCORES)), trace=trace)
    return _assemble(res.results), res


def kernel(**inputs) -> np.ndarray:
    out, _ = _run(inputs, trace=False)
    return out


# revision 35
# speedup vs baseline: 1.0363x; 1.0002x over previous
"""ConvLSTM (pixel-wise, 1x1 convs) Trainium2 Bass kernel.

Math (after exact algebraic folding):
  per pixel, per t:  g1 = W1x @ x_t + W1h @ h1 + b1   (W1x = Wih1 @ (W_red * denorm_scale))
                     i,f,g,o = split(g1); c1 = sig(f)*c1 + sig(i)*tanh(g); h1 = sig(o)*tanh(c1)
                     g2 = W21 @ h1 + W22 @ h2 + b2    (W21 = Wih2 @ Wc1)
                     c2,h2 analogous
  out = (W_head @ Wc2) @ h2_final + const

Sharding: batch b -> core b (8 cores, no collectives).

Per-core layout (one chunk = all 16384 pixels):
  S1e/S1o [128, HW] bf16 (alternating t): rows 0:64 h1, rows 64:92 x(t),
      row 92 ones (shipped as a 29th x channel in the DMA). One K=93
      matmul per gate computes W1h@h1 + W1x@x + b1; zero copies.
  S2 [128, HW] bf16: rows 64:128 h2. Cell2 gates accumulate two
      row-group-DISJOINT matmuls: W21 @ S1next[0:64] (groups 0,1) then
      W22 @ S2[64:128] (groups 2,3) -- LDWEIGHTS overlaps, no PE bubbles.
  c1/c2 [128, HALF]: A-half pixels on partitions 0:64, B-half on 64:128.
  Gates per cell land in ONE [128, 4F] PSUM tile in order (i, f, o, g).
  gmode=vtanh: sigma(x) = (tanh(x/2)+1)/2 with the 1/2 folded into the
      gate weights and doubled c/h state folded into consumer weights, so
      ONE Tanh instruction covers all four gates (ACT inst fixed cost is
      ~525 cyc; also bf16-safe -- only near-zero tanh values are stored,
      never sigmoid values near 0.5 whose bf16 ULP would swamp the tiny
      gate signals of this weight distribution).
  Pointwise runs on DVE via fused scalar_tensor_tensor ((a op s) op b);
  h is built once in a [128, F] plane then placed with two 4x-mode bf16
  copies. Emission is software-pipelined across slots (cell2 lagged) so
  ACT/DVE/PE queues stay fed.
  NOTE (hardware, verified by minimal repro): an accumulating matmul pair
  with K<=64 at row bases 0 and 64 into one PSUM region makes walrus pick
  row-tiling and faults the device -- cell2's first mm is padded to K=65.
  The PE also never leaves the HAM-throttled 1.2 GHz state in this
  environment (warmup bursts do not help), so matmul cost is ~N/1.2ns.
"""

import numpy as np
import ml_dtypes

import concourse.bass as bass
import concourse.tile as tile
from concourse import bacc, mybir
from concourse.bass_utils import run_bass_kernel_spmd

F32 = mybir.dt.float32
BF16 = mybir.dt.bfloat16
AF = mybir.ActivationFunctionType

T, CIN, HID = 8, 28, 64
H = W = 128
HW = H * W            # pixels per core (one batch element)
HALF = HW // 2
NCORES = 8
K1 = HID + CIN + 1    # 93: h1 rows, x rows, ones row

import os
CFG = dict(
    fd=512,            # pixels per half per group (psum: 2 cells x [128, 4*fd] fp32)
    c_dtype="f32",     # c-state dtype: "f32" | "bf16"
    plane_bufs=2,
    emit="pipe",       # "pipe" (software-pipelined slots) | "serial" (per-group)
    gmode="vtanh",     # "vtanh": sigma(x)=(tanh(x/2)+1)/2 -> ONE merged Tanh
                       # over all 4 gates, doubled c/h state folded into
                       # weights (bf16-safe: only tanh values are stored);
                       # "tanh": exact sigmoid+tanh on separate insts
    vstt=7,            # vtanh bisect bits: 1=h_muls STT, 2=dve_c STT, 4=merged act
    warmup=0,          # PE warmup matmul count (HAM clock-gate priming)
)
for _k in list(CFG):
    _v = os.environ.get(f"KCFG_{_k.upper()}")
    if _v is not None:
        CFG[_k] = int(_v) if _v.isdigit() else _v


def _fold_weights(inputs):
    """Host-side exact algebraic folding (all fp32 numpy)."""
    f = np.float32
    W_red = inputs["W_red"].astype(f)
    b_red = inputs["b_red"].astype(f)
    # de-normalization of channels 11 (u) and 12 (v), folded into W_red
    a = np.ones(CIN, f); a[11] = f(0.15); a[12] = f(0.12)
    d = np.zeros(CIN, f); d[11] = f(0.02); d[12] = f(-0.01)
    W_red_eff = W_red * a[None, :]
    b_red_eff = b_red + W_red @ d

    W1x = inputs["Wih1"].astype(f) @ W_red_eff          # [256, 28]
    W1h = inputs["Whh1"].astype(f)                      # [256, 64]
    b1 = (inputs["bih1"] + inputs["bhh1"]).astype(f) + inputs["Wih1"].astype(f) @ b_red_eff
    W21 = inputs["Wih2"].astype(f) @ inputs["Wc1"].astype(f)   # [256, 64]
    W22 = inputs["Whh2"].astype(f)                      # [256, 64]
    b2 = (inputs["bih2"] + inputs["bhh2"]).astype(f) + inputs["Wih2"].astype(f) @ inputs["bc1"].astype(f)
    whead = (inputs["W_head"].astype(f) @ inputs["Wc2"].astype(f))[0]     # [64]
    bhead = float((inputs["W_head"].astype(f) @ inputs["bc2"].astype(f) + inputs["b_head"].astype(f)).reshape(()))

    # reorder gate blocks (i, f, g, o) -> (i, f, o, g) so the three
    # sigmoid gates are contiguous in the PSUM tile
    perm = np.r_[0:64, 64:128, 192:256, 128:192]
    W1x, W1h, W21, W22 = W1x[perm], W1h[perm], W21[perm], W22[perm]
    b1, b2 = b1[perm], b2[perm]


    w1 = np.zeros((128, 256), f)
    w1[0:HID] = W1h.T
    w1[HID:HID + CIN] = W1x.T
    w1[HID + CIN] = b1
    w2a = np.zeros((128, 256), f)
    w2a[0:HID] = W21.T
    w2b = np.zeros((128, 256), f)
    w2b[HID:128] = W22.T
    # cell2 bias rides an extra K=29 matmul against [x; ones] only when
    # nonzero (it is zero for the reference weight distribution)
    w2c = None
    if np.any(b2 != 0):
        w2c = np.zeros((128, 256), f)
        w2c[HID + CIN] = b2
    wh = np.zeros((128, 1), f)
    wh[HID:, 0] = whead
    d = dict(w1=w1, w2a=w2a, w2b=w2b, wh=wh)
    if w2c is not None:
        d["w2c"] = w2c
    if CFG["gmode"] == "vtanh":
        # sigma(x) = (tanh(x/2) + 1)/2: halve the i,f,o gate pre-activations
        # (cols 0:192) so ONE Tanh inst covers all four gates; h is stored
        # doubled (h_hat = (tanh(o-pre)+1)*tanh(c)) so halve every weight row
        # that reads it; c is stored doubled too (ACT tanh(c) uses scale=0.5).
        for nm, M in d.items():
            if nm != "wh":
                M[:, 0:192] *= 0.5
        d["w1"][0:HID] *= 0.5     # rows reading h1_hat
        d["w2a"][0:HID] *= 0.5
        d["w2b"][HID:128] *= 0.5  # rows reading h2_hat
        d["wh"] = wh * 0.5
    return d, bhead


def build(nc, bhead, has_b2):
    fd = CFG["fd"]
    ngrp = HALF // fd
    NSLOT = T * ngrp
    st_dt = {"f32": F32, "bf16": BF16}[CFG["c_dtype"]]

    x_d = nc.dram_tensor("xt", [T, CIN + 1, HW], BF16, kind="ExternalInput").ap()
    w_names = ["w1", "w2a", "w2b", "wh"] + (["w2c"] if has_b2 else [])
    w_dram = {nm: nc.dram_tensor(nm, [128, 1] if nm == "wh" else [128, 256], F32,
                                 kind="ExternalInput").ap() for nm in w_names}
    # out[i, j] = pixel j*128 + i of this core's [H, W] map (host transposes)
    out_d = nc.dram_tensor("out", [128, HW // 128], F32, kind="ExternalOutput").ap()

    with tile.TileContext(nc) as tc:
        with (
            tc.tile_pool(name="const", bufs=1) as const,
            tc.tile_pool(name="state", bufs=1) as state,
            tc.tile_pool(name="planes", bufs=CFG["plane_bufs"]) as planes,
            tc.tile_pool(name="outp", bufs=1) as outp,
            tc.tile_pool(name="psum", bufs=1, space=bass.MemorySpace.PSUM) as psum,
        ):
            # Stage weights via fp32 tiles + one convert copy each, so every
            # matmul waits on a single compute producer (the fused LDWEIGHTS
            # has very few sync-wait slots).
            w_sb = {}
            for nm in w_names:
                shp = [128, 1] if nm == "wh" else [128, 256]
                wf = const.tile(shp, F32, tag=f"{nm}f")
                nc.sync.dma_start(wf[:], w_dram[nm])
                wb = const.tile(shp, BF16, tag=nm)
                nc.vector.tensor_copy(wb[:], wf[:])
                w_sb[nm] = wb

            S1e = state.tile([128, HW], BF16, tag="S1e")
            S1o = state.tile([128, HW], BF16, tag="S1o")
            Ss = [S1e, S1o]
            S2 = state.tile([128, HW], BF16, tag="S2")
            c1 = state.tile([128, HALF], st_dt, tag="c1")
            c2 = state.tile([128, HALF], st_dt, tag="c2")
            out_sb = outp.tile([128, HW // 128], F32, tag="osb")

            # PE warmup: ~4.5us of back-to-back matmuls so the PE_HAM clock
            # gate opens (cold 1.2 GHz -> warm 2.4 GHz) before the real work;
            # steady-state PE gaps stay under the ~3.4us re-throttle window.
            if CFG["warmup"]:
                wu = psum.tile([128, 4 * fd], F32, tag="P0", name="warm")
                for _ in range(CFG["warmup"]):
                    nc.tensor.matmul(wu[0:64, 0:128],
                                     w_sb["w1"][0:HID, 0:64],
                                     w_sb["w1"][0:HID, 0:128])

            # per-slot live tile handles (psum gate tiles + act output planes)
            P0s, P1s, pl1, pl2 = {}, {}, {}, {}

            def slot_tg(s):
                return s // ngrp, s % ngrp

            def cols(g):
                return g * fd, HALF + g * fd      # A-half / B-half col starts

            def c1_mms(s):
                if not (0 <= s < NSLOT):
                    return
                t, g = slot_tg(s)
                Scur = Ss[t % 2]
                if g == 0:
                    if t == 0:
                        nc.sync.dma_start(Scur[HID:K1, :], x_d[0])
                    if t + 1 < T:
                        nc.sync.dma_start(Ss[(t + 1) % 2][HID:K1, :], x_d[t + 1])
                a0, b0 = cols(g)
                ks = slice(0, K1) if t > 0 else slice(HID, K1)
                P = psum.tile([128, 4 * fd], F32, tag="P0", name="P0")
                P0s[s] = P
                for q in range(4):
                    for (cb, po) in ((a0, 0), (b0, 64)):
                        nc.tensor.matmul(
                            P[po:po + 64, q * fd:(q + 1) * fd],
                            w_sb["w1"][ks, q * 64:(q + 1) * 64],
                            Scur[ks, cb:cb + fd],
                        )

            def act_gates(s, Ps, pl, tagp):
                if not (0 <= s < NSLOT):
                    return
                P = Ps.pop(s)
                if CFG["gmode"] == "vtanh":
                    sfo = planes.tile([128, 4 * fd], BF16, tag=f"sfo{tagp}")
                    if CFG["vstt"] & 4:
                        nc.scalar.activation(sfo[:], P[:], AF.Tanh)
                    else:
                        nc.scalar.activation(sfo[:, 0:3 * fd], P[:, 0:3 * fd], AF.Tanh)
                        nc.scalar.activation(sfo[:, 3 * fd:4 * fd], P[:, 3 * fd:4 * fd], AF.Tanh)
                    pl[s] = (sfo, sfo[:, 3 * fd:4 * fd])
                else:
                    sfo = planes.tile([128, 3 * fd], BF16, tag=f"sfo{tagp}")
                    tgp = planes.tile([128, fd], BF16, tag=f"tg{tagp}")
                    nc.scalar.activation(sfo[:], P[:, 0:3 * fd], AF.Sigmoid)
                    nc.scalar.activation(tgp[:], P[:, 3 * fd:4 * fd], AF.Tanh)
                    pl[s] = (sfo, tgp[:])

            def act1(s):
                act_gates(s, P0s, pl1, "1")

            def dve_c(s, pl, cc, tagp):
                # gmode=vtanh (v* = tanh(pre/2), c_hat = 2c):
                #   c_hat = (v_f+1)*c_hat*0.5 + (v_i+1)*tanh(g)
                # gmode=tanh: c = sig(f)*c + sig(i)*tanh(g)
                if not (0 <= s < NSLOT):
                    return
                t, g = slot_tg(s)
                sfo, tgp = pl[s]
                cg = slice(g * fd, (g + 1) * fd)
                si = sfo[:, 0:fd]
                sf = sfo[:, fd:2 * fd]
                Alu = mybir.AluOpType
                vt = CFG["gmode"] == "vtanh"
                if vt:
                    if CFG["vstt"] & 2:
                        if t > 0:
                            p = planes.tile([128, fd], BF16, tag=f"t2{tagp}")
                            q = planes.tile([128, fd], st_dt, tag=f"t1{tagp}")
                            nc.vector.scalar_tensor_tensor(
                                p[:], si, 1.0, tgp, Alu.add, Alu.mult)
                            nc.vector.scalar_tensor_tensor(
                                q[:], sf, 1.0, cc[:, cg], Alu.add, Alu.mult)
                            nc.vector.scalar_tensor_tensor(
                                cc[:, cg], q[:], 0.5, p[:], Alu.mult, Alu.add)
                        else:
                            nc.vector.scalar_tensor_tensor(
                                cc[:, cg], si, 1.0, tgp, Alu.add, Alu.mult)
                    else:
                        p = planes.tile([128, fd], BF16, tag=f"t2{tagp}")
                        ti = planes.tile([128, fd], BF16, tag=f"ti{tagp}")
                        nc.vector.tensor_scalar_add(ti[:], si, 1.0)
                        nc.vector.tensor_mul(p[:], ti[:], tgp)
                        if t > 0:
                            q = planes.tile([128, fd], st_dt, tag=f"t1{tagp}")
                            tf = planes.tile([128, fd], BF16, tag=f"tf{tagp}")
                            nc.vector.tensor_scalar_add(tf[:], sf, 1.0)
                            nc.vector.tensor_mul(q[:], tf[:], cc[:, cg])
                            qh = planes.tile([128, fd], st_dt, tag=f"qh{tagp}")
                            nc.vector.tensor_scalar_mul(qh[:], q[:], 0.5)
                            nc.vector.tensor_add(cc[:, cg], qh[:], p[:])
                        else:
                            nc.vector.tensor_copy(cc[:, cg], p[:])
                    return
                t2 = planes.tile([128, fd], BF16, tag=f"t2{tagp}")
                if t > 0:
                    t1 = planes.tile([128, fd], st_dt, tag=f"t1{tagp}")
                    nc.vector.tensor_mul(t2[:], si, tgp)
                    nc.vector.tensor_mul(t1[:], sf, cc[:, cg])
                    nc.vector.tensor_add(cc[:, cg], t1[:], t2[:])
                else:
                    nc.vector.tensor_mul(cc[:, cg], si, tgp)

            def act_tc(s, pl, cc, tagp):
                if not (0 <= s < NSLOT):
                    return
                t, g = slot_tg(s)
                cg = slice(g * fd, (g + 1) * fd)
                tch = planes.tile([128, fd], BF16, tag=f"tch{tagp}")
                scl = 0.5 if CFG["gmode"] == "vtanh" else 1.0
                nc.scalar.activation(tch[:], cc[:, cg], AF.Tanh, scale=scl)
                pl[s] = (pl[s][0], tch)

            def h_muls(s, pl, dst_tile_fn, dst_rows, tagp):
                # h = sig(o)*tanh(c); vtanh stores h_hat = (v_o+1)*tanh(c) = 2h
                if not (0 <= s < NSLOT):
                    return
                t, g = slot_tg(s)
                sfo, tch = pl.pop(s)
                a0, b0 = cols(g)
                dst = dst_tile_fn(t)
                Alu = mybir.AluOpType
                if CFG["gmode"] == "vtanh":
                    # one full-width STT then two 4x-mode bf16 copies into the
                    # state rows: 1 x 657ns + 2 x ~195ns beats 2 x 657ns STTs
                    hp = planes.tile([128, fd], BF16, tag=f"hp{tagp}")
                    nc.vector.scalar_tensor_tensor(
                        hp[:], sfo[:, 2 * fd:3 * fd], 1.0, tch[:],
                        Alu.add, Alu.mult)
                    nc.vector.tensor_copy(dst[dst_rows, a0:a0 + fd], hp[0:64, :])
                    nc.vector.tensor_copy(dst[dst_rows, b0:b0 + fd], hp[64:128, :])
                else:
                    for (po, cb) in ((0, a0), (64, b0)):
                        so = sfo[po:po + 64, 2 * fd:3 * fd]
                        nc.vector.tensor_mul(dst[dst_rows, cb:cb + fd],
                                             so, tch[po:po + 64, :])

            def c2_mms(s):
                if not (0 <= s < NSLOT):
                    return
                t, g = slot_tg(s)
                Snxt = Ss[(t + 1) % 2]
                a0, b0 = cols(g)
                P = psum.tile([128, 4 * fd], F32, tag="P1", name="P1")
                P1s[s] = P
                # Per gate: K=65 rows 0:65 (h1 + a zero weight row) opens the
                # accumulation, K=64 rows 64:128 (h2) closes it. The K=65 pad
                # is deliberate: a K<=64 pair with bases 0 and 64 into one
                # PSUM region makes walrus pick row-tiling mode, and a row-
                # tiled mm accumulating onto another row tile's output faults
                # the HW (minimal repro verified). K only shifts the drain, so
                # the pad is free.
                # Waves: all four "a" mms stream back-to-back (same rows, so
                # LDWEIGHTS pulls ahead) on alternating column halves (col-
                # tiled pairs run concurrently), then the "b" wave closes the
                # groups; a second wave pass covers the swapped halves. Only
                # one accumulation group is ever pending per PSUM bank.
                halves = ((a0, 0), (b0, 64))
                for wave in range(2):
                    for q in range(4):
                        cb, po = halves[(q + wave) % 2]
                        nc.tensor.matmul(
                            P[po:po + 64, q * fd:(q + 1) * fd],
                            w_sb["w2a"][0:HID + 1, q * 64:(q + 1) * 64],
                            Snxt[0:HID + 1, cb:cb + fd],
                            start=True, stop=(t == 0 and not has_b2),
                        )
                    for q in range(4):
                        cb, po = halves[(q + wave) % 2]
                        if t > 0:
                            nc.tensor.matmul(
                                P[po:po + 64, q * fd:(q + 1) * fd],
                                w_sb["w2b"][HID:128, q * 64:(q + 1) * 64],
                                S2[HID:128, cb:cb + fd],
                                start=False, stop=not has_b2,
                            )
                        if has_b2:
                            nc.tensor.matmul(
                                P[po:po + 64, q * fd:(q + 1) * fd],
                                w_sb["w2c"][HID:K1, q * 64:(q + 1) * 64],
                                Snxt[HID:K1, cb:cb + fd],
                                start=False, stop=True,
                            )

            def act2(s):
                act_gates(s, P1s, pl2, "2")

            s1next = lambda t: Ss[(t + 1) % 2]
            s2tile = lambda t: S2

            if CFG["emit"] == "pipe":
                # Software pipeline, cell2 lagged two slots behind cell1's
                # matmuls: C2(s-1) runs at slot-s start with h1(s-1) already
                # computed last slot, so the PE never waits on the
                # act->dve->tanh->h chain inside a slot.
                c1_mms(0)
                for s in range(NSLOT + 2):
                    act1(s)
                    c1_mms(s + 1)
                    dve_c(s, pl1, c1, "a")
                    act2(s - 2)
                    act_tc(s, pl1, c1, "a")
                    dve_c(s - 2, pl2, c2, "b")
                    h_muls(s, pl1, s1next, slice(0, HID), 'a')
                    act_tc(s - 2, pl2, c2, "b")
                    c2_mms(s - 1)
                    h_muls(s - 2, pl2, s2tile, slice(HID, 128), 'b')
            else:
                for s in range(NSLOT):
                    c1_mms(s)
                    act1(s)
                    dve_c(s, pl1, c1, "a")
                    act_tc(s, pl1, c1, "a")
                    h_muls(s, pl1, s1next, slice(0, HID), 'a')
                    c2_mms(s)
                    act2(s)
                    dve_c(s, pl2, c2, "b")
                    act_tc(s, pl2, c2, "b")
                    h_muls(s, pl2, s2tile, slice(HID, 128), 'b')

            # head: out[pix] = whead @ h2[pix] + bhead, pixels as matmul M-dim
            ncols = HW // 128
            ph = psum.tile([128, ncols], F32, tag="P0", name="ph")
            for j in range(ncols):
                nc.tensor.matmul(
                    ph[:, j:j + 1],
                    S2[HID:128, j * 128:(j + 1) * 128],
                    w_sb["wh"][HID:128, 0:1],
                )
            nc.vector.tensor_scalar_add(out_sb[:], ph[:], float(bhead))
            nc.sync.dma_start(out_d, out_sb[:])
    nc.compile()
    return nc


def _make_nc():
    # Bacc (not raw Bass): its compile() runs move_matmul_waits_to_ldweights +
    # generate_event_semaphores, required to satisfy TRN2's 1-wait-per-inst limit.
    return bacc.Bacc("TRN2", target_bir_lowering=False, debug=False,
                     num_devices=NCORES, enable_partition_id=False)


def _in_maps(inputs):
    folded, _ = _fold_weights(inputs)
    x = np.asarray(inputs["x"], dtype=np.float32)
    x_bf = x.reshape(NCORES, T, CIN, HW).astype(ml_dtypes.bfloat16)
    ones = np.ones((T, 1, HW), ml_dtypes.bfloat16)
    maps = []
    for b in range(NCORES):
        m = dict(folded)
        m["xt"] = np.ascontiguousarray(
            np.concatenate([x_bf[b], ones], axis=1))
        maps.append(m)
    return maps


def _assemble(results):
    out = np.empty((NCORES, H, W), np.float32)
    for b in range(NCORES):
        o = results[b]["out"]          # [128, HW//128], o[i, j] = pixel j*128+i
        out[b] = o.T.reshape(H, W)
    return out


def _run(inputs, trace=False):
    folded, bhead = _fold_weights(inputs)
    nc = build(_make_nc(), bhead, "w2c" in folded)
    maps = _in_maps(inputs)
    res = run_bass_kernel_spmd(nc, maps, core_ids=list(range(NCORES)), trace=trace)
    return _assemble(res.results), res


def kernel(**inputs) -> np.ndarray:
    out, _ = _run(inputs, trace=False)
    return out
